# revision 27
# baseline (speedup 1.0000x reference)
"""LLaMA causal self-attention, 8-way head-tensor-parallel Trainium2 Bass kernel.

Sharding: each of 8 cores computes 4 query heads + its 1 KV head-group
(Wq/Wk/Wv column-sharded), plus a row-shard of Wo producing a partial
(S, DIM) output; partials are summed on the host (the all-reduce of the
row-sharded Wo matmul).

v2 layout notes (vs the f32r baseline):
  - All matmul operands are fp16 (1 cycle/row on the PE vs ~2 for f32r),
    halving both PE time and HBM read bytes. PSUM accumulation stays f32.
  - Exact-causal column trimming: for diagonal key-tiles only columns
    >= c0 = 128*(j-4t) of the q-tile are computed (scores/exp/PV), and the
    causal mask multiply shrinks to a single shared [128,128] step band.
  - Scores for a head-pair go into one [128,2,512] PSUM tile -> ONE exp
    instruction over both banks (amortizes Act access latency).
  - Softmax normalize: reciprocal of the ones-row sums read straight from
    PSUM, rank-1 f32r matmul broadcast into spare scps banks, one DVE
    multiply (PSUM x PSUM -> fp16 SBUF). No staging copies.
  - Out-projection PSUM banks are DMA'd directly to HBM (f32 partial),
    split across the SP and GpSimd queues; no SBUF staging.
  - V transpose via 16-bit DMA-transpose (XBAR) instead of PE+PSUM.
  - Startup DMAs are spread across SP/DVE/Act/Pool queues so the first
    projection matmul starts ~7us in instead of ~42us.
"""

import numpy as np
import ml_dtypes  # noqa: F401  (registers bfloat16 numpy dtype)

import concourse.bass as bass
import concourse.mybir as mybir
import concourse.tile as tile
from contextlib import nullcontext
from concourse import bacc
from concourse.bass import ts, ds
from concourse.bass_utils import run_bass_kernel_spmd

F32 = mybir.dt.float32
F32R = mybir.dt.float32r
F16 = mybir.dt.float16

S = 2048
DIM = 2048
H = 32
KVH = 8
D = 64
NCORES = 8
HQ = H // NCORES          # 4 q heads per core
CQ = HQ * D               # 256 q cols per core
ST = 512                  # s-tile width in QKV projection phase
QT = 512                  # q-tile width in attention
NKT = S // 128            # 16 key tiles
NDT = DIM // 128          # 16 contraction tiles for projections
NST = S // ST             # 4 projection s-tiles
NQT = S // QT             # 4 attention q-tiles


def r(ap):
    return ap.bitcast(F32R)


def _build(causal: bool, use_mask: bool, dump: bool = False):
    nc = bacc.Bacc("TRN2", target_bir_lowering=False, debug=False,
                   num_devices=NCORES, name="llama_attn")
    if dump:
        qTd = nc.dram_tensor("qTd", [2, 128, S], F16, kind="ExternalOutput")
        kTd = nc.dram_tensor("kTd", [128, S], F16, kind="ExternalOutput")
        vd = nc.dram_tensor("vd", [128, NKT, 65], F16, kind="ExternalOutput")
        attnd = nc.dram_tensor("attnd", [2, NQT, 128, QT], F16,
                               kind="ExternalOutput")
    # all big operands come pre-swizzled from the host so every DMA is
    # per-partition contiguous (128 descriptors instead of thousands)
    xT = nc.dram_tensor("xT", [NST, 128, NDT * ST], F16, kind="ExternalInput")
    wq = nc.dram_tensor("wq", [128, NDT * CQ], F16, kind="ExternalInput")
    wkv = nc.dram_tensor("wkv", [128, NDT * 128], F16, kind="ExternalInput")
    wo = nc.dram_tensor("wo", [128, 2 * DIM], F16, kind="ExternalInput")
    bqd = nc.dram_tensor("bq", [CQ], F32, kind="ExternalInput")
    bkvd = nc.dram_tensor("bkv", [128], F32, kind="ExternalInput")
    ccd = nc.dram_tensor("cc", [128, S], F16, kind="ExternalInput")
    ssd = nc.dram_tensor("ssgn", [128, S], F16, kind="ExternalInput")
    trid = nc.dram_tensor("triband", [128, 2, 128], F16, kind="ExternalInput")
    onehd = nc.dram_tensor("onesh", [128], F16, kind="ExternalInput")
    if use_mask:
        maskt = nc.dram_tensor("maskt", [S, S], F16, kind="ExternalInput")
    partial = nc.dram_tensor("partial", [S, DIM], F16, kind="ExternalOutput")

    with tile.TileContext(nc) as tc:
        with tc.tile_pool(name="persist", bufs=1) as pp:
            # --- persistent inputs, spread across DMA queues ---
            # sync: biases then the x stream (phase-1 critical path)
            bq_sb = pp.tile([128, 2], F32)
            nc.sync.dma_start(bq_sb[:], bqd.ap().rearrange("(t p) -> p t", p=128))
            bkv_sb = pp.tile([128, 1], F32)
            nc.sync.dma_start(bkv_sb[:], bkvd.ap()[:, None])
            # sync: projection weights (needed by the very first matmuls)
            wq_sb = pp.tile([128, NDT, CQ], F16)
            nc.sync.dma_start(wq_sb[:], wq.ap().rearrange("p (a c) -> p a c", c=CQ))
            wkv_sb = pp.tile([128, NDT, 128], F16)
            nc.sync.dma_start(wkv_sb[:], wkv.ap().rearrange("p (a c) -> p a c", c=128))
            # gpsimd: rope tables + small constants (needed ~10us in)
            cc_sb = pp.tile([128, S], F16)
            nc.gpsimd.dma_start(cc_sb[:], ccd[:])
            ss_sb = pp.tile([128, S], F16)
            nc.gpsimd.dma_start(ss_sb[:], ssd[:])
            tri_sb = pp.tile([128, 2, 128], F16)
            nc.gpsimd.dma_start(tri_sb[:], trid[:])
            ones65 = pp.tile([65, 64], F16, name="ones65")
            nc.gpsimd.dma_start(ones65[64:65, :], onehd.ap()[None, 0:64])
            # scalar: out-projection weights (phase 2 only)
            wo_sb = pp.tile([128, 2, DIM], F16)
            nc.scalar.dma_start(wo_sb[:], wo.ap().rearrange("p (a e) -> p a e", e=DIM))
            if use_mask:
                mask_sb = pp.tile([128, NKT, S], F16)
                nc.scalar.dma_start(mask_sb[:],
                                    maskt.ap().rearrange("(a p) q -> p a q", p=128))

            # persistent activations
            qT = [pp.tile([128, S], F16, tag=f"qt{i}", name=f"qt{i}") for i in range(2)]
            kT = pp.tile([128, S], F16)           # rows 0:64 and 64:128 both = roped k
            v_sb = pp.tile([128, NKT, 65], F16)   # [kpos, ktile, ch + ones]
            nc.gpsimd.dma_start(v_sb[:, :, 64:65],
                                onehd.ap()[:, None, None].to_broadcast((128, NKT, 1)))
            attn = [[pp.tile([128, QT], F16, tag=f"attn{c}_{t}", name=f"attn{c}_{t}")
                     for t in range(NQT)] for c in range(2)]

            # ------- Phase 1: QKV projections + fused RoPE + v transpose -------
            with tc.tile_pool(name="xstream", bufs=2) as xp, \
                 tc.tile_pool(name="qkvps", bufs=2, space="PSUM") as qps, \
                 tc.tile_pool(name="rawp", bufs=2) as rawp, \
                 tc.tile_pool(name="vstg", bufs=3) as vstg, \
                 tc.tile_pool(name="rope", bufs=3) as rp:

                def rope(raw_ap, out_ap, rows, ssl, swq):
                    sw = rp.tile([128, ST], F16, tag="ropesw")
                    t1 = rp.tile([128, ST], F16, tag="ropet1")
                    for b in rows:
                        swq.dma_start(sw[b:b + 32, :], raw_ap[b + 32:b + 64, :])
                        swq.dma_start(sw[b + 32:b + 64, :], raw_ap[b:b + 32, :])
                    lo, hi = rows[0], rows[-1] + 64
                    nc.vector.tensor_mul(t1[lo:hi, :], raw_ap[lo:hi, :], cc_sb[lo:hi, ssl])
                    nc.vector.tensor_mul(sw[lo:hi, :], sw[lo:hi, :], ss_sb[lo:hi, ssl])
                    nc.vector.tensor_add(out_ap[lo:hi, ssl], t1[lo:hi, :], sw[lo:hi, :])

                for st in range(NST):
                    ssl = ts(st, ST)
                    xt = xp.tile([128, NDT, ST], F16, tag="xt")
                    xr = xT.ap()[st].rearrange("p (a s) -> p a s", s=ST)
                    for half in range(2):
                        nc.sync.dma_start(xt[:, ts(half, 8), :], xr[:, ts(half, 8), :])
                    pq0 = qps.tile([128, ST], F32, tag="q0")
                    pq1 = qps.tile([128, ST], F32, tag="q1")
                    pkv = qps.tile([128, ST], F32, tag="kv")
                    for kt in range(NDT):
                        st_flag, sp_flag = kt == 0, kt == NDT - 1
                        nc.tensor.matmul(pq0[:], wq_sb[:, kt, 0:128], xt[:, kt, :],
                                         start=st_flag, stop=sp_flag)
                        nc.tensor.matmul(pq1[:], wq_sb[:, kt, 128:256], xt[:, kt, :],
                                         start=st_flag, stop=sp_flag)
                        nc.tensor.matmul(pkv[:], wkv_sb[:, kt, :], xt[:, kt, :],
                                         start=st_flag, stop=sp_flag)
                    q0_raw = rawp.tile([128, ST], F16, tag="q0r")
                    q1_raw = rawp.tile([128, ST], F16, tag="q1r")
                    kv_raw = rawp.tile([128, ST], F16, tag="kvr")
                    nc.scalar.activation(q0_raw[:], pq0[:],
                                         mybir.ActivationFunctionType.Identity,
                                         bias=bq_sb[:, 0:1])
                    nc.scalar.activation(q1_raw[:], pq1[:],
                                         mybir.ActivationFunctionType.Identity,
                                         bias=bq_sb[:, 1:2])
                    nc.scalar.activation(kv_raw[:], pkv[:],
                                         mybir.ActivationFunctionType.Identity,
                                         bias=bkv_sb[:, 0:1])
                    rope(q0_raw[:], qT[0][:], [0, 64], ssl, nc.gpsimd)
                    rope(q1_raw[:], qT[1][:], [0, 64], ssl, nc.scalar)
                    rope(kv_raw[:], kT[:], [64], ssl, nc.sync)
                    nc.sync.dma_start(kT[0:64, ssl], kT[64:128, ssl])
                    for vc in range(ST // 128):
                        j = (st * ST) // 128 + vc
                        # XBAR transpose misaddresses offset outputs; stage at
                        # offset 0 then copy into the strided v slot
                        stg = vstg.tile([128, 64], F16, tag="stg")
                        nc.scalar.dma_start(stg[:], kv_raw[0:64, ts(vc, 128)],
                                            transpose=True)
                        nc.vector.tensor_copy(v_sb[:, j, 0:64], stg[:])

            # ------- Phase 2+3 fused: attention with interleaved out-proj -------
            with tc.tile_pool(name="scps", bufs=2, space="PSUM") as scps, \
                 tc.tile_pool(name="avps", bufs=2, space="PSUM") as avps, \
                 tc.tile_pool(name="opsp", bufs=2, space="PSUM") as opsp, \
                 tc.tile_pool(name="ptp", bufs=3) as ptp, \
                 tc.tile_pool(name="nrm", bufs=2) as nrm, \
                 tc.tile_pool(name="osb", bufs=4) as osb:
                for t in range(NQT):
                    tsl = ts(t, QT)
                    n_k = 4 * (t + 1) if causal else NKT
                    for hp in range(2):
                        aps = [avps.tile([65, QT], F32, tag="av", name=f"av{t}_{hp}_{hh}")
                               for hh in range(2)]
                        pts = {}
                        c0s = {}
                        for j in range(n_k):
                            jsl = ts(j, 128)
                            c0 = 128 * (j - 4 * t) if (causal and j >= 4 * t) else 0
                            c0s[j] = c0
                            sc = scps.tile([128, 2, QT], F32, tag="sc", name="sc")
                            for half, base in ((0, 0), (1, 64)):
                                nc.tensor.matmul(sc[:, half, c0:],
                                                 kT[base:base + 64, jsl],
                                                 qT[hp][base:base + 64, tsl][:, c0:],
                                                 start=True, stop=True)
                            pt = ptp.tile([128, 2, QT], F16, tag="pt")
                            nc.scalar.activation(pt[:, :, c0:], sc[:, :, c0:],
                                                 mybir.ActivationFunctionType.Exp,
                                                 scale=0.125)
                            if causal and j >= 4 * t:
                                nc.gpsimd.tensor_mul(pt[:, :, c0:c0 + 128],
                                                     pt[:, :, c0:c0 + 128],
                                                     tri_sb[:])
                            if use_mask:
                                for hh in range(2):
                                    nc.vector.tensor_mul(pt[:, hh, :], pt[:, hh, :],
                                                         mask_sb[:, j, tsl])
                            pts[j] = pt
                            # PV one j behind scores keeps the PE fed during exp
                            if j > 0:
                                jp = j - 1
                                for hh in range(2):
                                    nc.tensor.matmul(aps[hh][:, c0s[jp]:],
                                                     v_sb[:, jp, :],
                                                     pts[jp][:, hh, c0s[jp]:],
                                                     start=(jp == 0), stop=False)
                                del pts[jp]
                        jp = n_k - 1
                        for hh in range(2):
                            nc.tensor.matmul(aps[hh][:, c0s[jp]:], v_sb[:, jp, :],
                                             pts[jp][:, hh, c0s[jp]:],
                                             start=(jp == 0), stop=True)
                        # softmax normalize: 1/rowsum broadcast via rank-1 matmul
                        bcp = scps.tile([128, 2, QT], F32, tag="sc", name="bc")
                        for hh in range(2):
                            avsb = nrm.tile([65, QT], F32, tag="avsb")
                            nc.vector.tensor_copy(avsb[:], aps[hh][:])
                            # NB: single-partition custom-DVE ops silently
                            # no-op; run on the full 65-partition tile
                            rc = nrm.tile([65, QT], F32, tag="rc")
                            nc.vector.reciprocal_approx_fast(rc[:, :],
                                                             avsb[:, :])
                            rch = nrm.tile([65, QT], F16, tag="rch")
                            nc.vector.tensor_copy(rch[64:65, :], rc[64:65, :])
                            nc.tensor.matmul(bcp[0:64, hh, :], ones65[64:65, :],
                                             rch[64:65, :], start=True, stop=True)
                            if hh == 0:
                                nc.vector.tensor_mul(attn[hp][t][0:64, :],
                                                     avsb[0:64, :],
                                                     bcp[0:64, hh, :])
                            else:
                                tb = nrm.tile([64, QT], F16, tag="tb")
                                nc.vector.tensor_mul(tb[:], avsb[0:64, :],
                                                     bcp[0:64, hh, :])
                                nc.gpsimd.dma_start(attn[hp][t][64:128, :], tb[:])
                    # out-projection rows for this t
                    for sl in range(4):
                        ssub = 4 * t + sl
                        for et in range(4):
                            pps = opsp.tile([128, 512], F32, tag="op")
                            for ct in range(2):
                                nc.tensor.matmul(pps[:], attn[ct][t][:, ts(sl, 128)],
                                                 wo_sb[:, ct, ts(et, 512)],
                                                 start=(ct == 0), stop=(ct == 1))
                            ot = osb.tile([128, 512], F16, tag="ot")
                            if (4 * sl + et) % 3 == 2:
                                nc.scalar.activation(
                                    ot[:], pps[:],
                                    mybir.ActivationFunctionType.Copy)
                            else:
                                nc.vector.tensor_copy(ot[:], pps[:])
                            nc.sync.dma_start(partial[ts(ssub, 128), ts(et, 512)], ot[:])
                if dump:
                    for i in range(2):
                        nc.sync.dma_start(qTd[i], qT[i][:])
                    nc.sync.dma_start(kTd[:], kT[:])
                    nc.sync.dma_start(vd[:], v_sb[:])
                    for c in range(2):
                        for t in range(NQT):
                            nc.sync.dma_start(attnd[c, t], attn[c][t][:])

    nc.compile()
    return nc


_CACHE = {}
TRACE = False
LAST_EXEC_NS = None
LAST_RES = None


def _get(causal, use_mask):
    key = (causal, use_mask)
    if key not in _CACHE:
        _CACHE[key] = _build(causal, use_mask)
    return _CACHE[key]


def _perm_eo(w):
    # de-interleave channel pairs per 64-col head block: [evens, odds]
    cols = np.concatenate([np.arange(0, 64, 2), np.arange(1, 64, 2)])
    return w[..., cols]


def kernel(**inputs):
    x = np.asarray(inputs["x"], dtype=np.float32)
    fc = np.asarray(inputs["freqs_cos"], dtype=np.float32)
    fs = np.asarray(inputs["freqs_sin"], dtype=np.float32)
    mask = np.asarray(inputs["mask"])
    Wq = np.asarray(inputs["Wq"], dtype=np.float32)
    bq = np.asarray(inputs["bq"], dtype=np.float32)
    Wk = np.asarray(inputs["Wk"], dtype=np.float32)
    bk = np.asarray(inputs["bk"], dtype=np.float32)
    Wv = np.asarray(inputs["Wv"], dtype=np.float32)
    bv = np.asarray(inputs["bv"], dtype=np.float32)
    Wo = np.asarray(inputs["Wo"], dtype=np.float32)
    bo = np.asarray(inputs["bo"], dtype=np.float32)

    m2 = mask.reshape(S, S)
    if (m2 == 1).all():
        causal, use_mask = False, False
    elif np.array_equal(m2 != 0, np.tril(np.ones((S, S), dtype=bool))):
        causal, use_mask = True, False
    else:
        causal, use_mask = False, True
    nc = _get(causal, use_mask)

    f16 = np.float16

    def swiz(w, inner):
        # [A*128, inner] -> [128, A*inner]: row a*128+p becomes (p, a)
        a = w.shape[0] // 128
        return np.ascontiguousarray(
            w.reshape(a, 128, inner).transpose(1, 0, 2).reshape(128, a * inner),
            dtype=f16)

    # x: [DIM, S] -> [NST, 128, NDT*ST] with per-partition contiguous s-tiles
    xTf = x[0].T.astype(f16)              # (DIM, S)
    xT = np.ascontiguousarray(
        xTf.reshape(NDT, 128, NST, ST).transpose(2, 1, 0, 3).reshape(
            NST, 128, NDT * ST))
    cosT = fc.T  # (32, S)
    sinT = fs.T
    cc = np.ascontiguousarray(np.tile(cosT, (4, 1)), dtype=f16)
    ssgn = np.ascontiguousarray(
        np.concatenate([-sinT, sinT, -sinT, sinT], axis=0), dtype=f16)
    kl = np.arange(128)[:, None]
    qq = np.arange(128)[None, :]
    band = (qq >= kl).astype(f16)                  # [128,128] step
    tri = np.ascontiguousarray(
        np.broadcast_to(band[:, None, :], (128, 2, 128)), dtype=f16)

    Wq_h = Wq.reshape(DIM, H, D)
    bq_h = bq.reshape(H, D)
    Wk_h = Wk.reshape(DIM, KVH, D)
    bk_h = bk.reshape(KVH, D)

    in_maps = []
    for c in range(NCORES):
        hs = slice(HQ * c, HQ * (c + 1))
        wq_c = _perm_eo(Wq_h[:, hs, :]).reshape(DIM, CQ)
        bq_c = _perm_eo(bq_h[hs, :]).reshape(CQ)
        wk_c = _perm_eo(Wk_h[:, c, :])
        bk_c = _perm_eo(bk_h[c, :])
        wv_c = Wv[:, 64 * c:64 * (c + 1)]
        bv_c = bv[64 * c:64 * (c + 1)]
        wkv_c = np.concatenate([wv_c, wk_c], axis=1)
        bkv_c = np.concatenate([bv_c, bk_c]).astype(np.float32)
        wo_c = Wo[CQ * c:CQ * (c + 1), :]
        im = {
            "xT": xT, "wq": swiz(wq_c, CQ),
            "wkv": swiz(wkv_c, 128), "wo": swiz(wo_c, DIM),
            "bq": np.ascontiguousarray(bq_c, dtype=np.float32),
            "bkv": np.ascontiguousarray(bkv_c), "cc": cc,
            "ssgn": ssgn, "triband": tri,
            "onesh": np.ones(128, dtype=f16),
        }
        if use_mask:
            im["maskt"] = np.ascontiguousarray(m2.T.astype(f16))
        in_maps.append(im)

    global LAST_EXEC_NS, LAST_RES
    res = run_bass_kernel_spmd(nc, in_maps, core_ids=list(range(NCORES)), trace=TRACE)
    LAST_EXEC_NS = res.exec_time_ns
    LAST_RES = res
    out = np.zeros((S, DIM), dtype=np.float32)
    for rr in res.results:
        out += rr["partial"].astype(np.float32)
    out += bo
    return out.reshape(1, S, DIM)


# revision 33
# speedup vs baseline: 1.1256x; 1.1256x over previous
"""LLaMA causal self-attention, 8-way head-tensor-parallel Trainium2 Bass kernel.

Sharding: each of 8 cores computes 4 query heads + its 1 KV head-group
(Wq/Wk/Wv column-sharded), plus a row-shard of Wo producing a partial
(S, DIM) output; partials are summed on the host (the all-reduce of the
row-sharded Wo matmul).

v2 layout notes (vs the f32r baseline):
  - All matmul operands are fp16 (1 cycle/row on the PE vs ~2 for f32r),
    halving both PE time and HBM read bytes. PSUM accumulation stays f32.
  - Exact-causal column trimming: for diagonal key-tiles only columns
    >= c0 = 128*(j-4t) of the q-tile are computed (scores/exp/PV), and the
    causal mask multiply shrinks to a single shared [128,128] step band.
  - Scores for a head-pair go into one [128,2,512] PSUM tile -> ONE exp
    instruction over both banks (amortizes Act access latency).
  - Softmax normalize: reciprocal of the ones-row sums read straight from
    PSUM, rank-1 f32r matmul broadcast into spare scps banks, one DVE
    multiply (PSUM x PSUM -> fp16 SBUF). No staging copies.
  - Out-projection PSUM banks are DMA'd directly to HBM (f32 partial),
    split across the SP and GpSimd queues; no SBUF staging.
  - V transpose via 16-bit DMA-transpose (XBAR) instead of PE+PSUM.
  - Startup DMAs are spread across SP/DVE/Act/Pool queues so the first
    projection matmul starts ~7us in instead of ~42us.
"""

import numpy as np
import ml_dtypes  # noqa: F401  (registers bfloat16 numpy dtype)

import concourse.bass as bass
import concourse.mybir as mybir
import concourse.tile as tile
from contextlib import nullcontext
from concourse import bacc
from concourse.bass import ts, ds
from concourse.bass_utils import run_bass_kernel_spmd

F32 = mybir.dt.float32
F32R = mybir.dt.float32r
F16 = mybir.dt.float16

S = 2048
DIM = 2048
H = 32
KVH = 8
D = 64
NCORES = 8
HQ = H // NCORES          # 4 q heads per core
CQ = HQ * D               # 256 q cols per core
ST = 512                  # s-tile width in QKV projection phase
QT = 512                  # q-tile width in attention
NKT = S // 128            # 16 key tiles
NDT = DIM // 128          # 16 contraction tiles for projections
NST = S // ST             # 4 projection s-tiles
NQT = S // QT             # 4 attention q-tiles


def r(ap):
    return ap.bitcast(F32R)


def _build(causal: bool, use_mask: bool, dump: bool = False):
    nc = bacc.Bacc("TRN2", target_bir_lowering=False, debug=False,
                   num_devices=NCORES, name="llama_attn")
    if dump:
        qTd = nc.dram_tensor("qTd", [2, 128, S], F16, kind="ExternalOutput")
        kTd = nc.dram_tensor("kTd", [128, S], F16, kind="ExternalOutput")
        vd = nc.dram_tensor("vd", [128, NKT, 65], F16, kind="ExternalOutput")
        attnd = nc.dram_tensor("attnd", [2, NQT, 128, QT], F16,
                               kind="ExternalOutput")
    # all big operands come pre-swizzled from the host so every DMA is
    # per-partition contiguous (128 descriptors instead of thousands)
    xT = nc.dram_tensor("xT", [NST, 128, NDT * ST], F16, kind="ExternalInput")
    wq = nc.dram_tensor("wq", [128, NDT * CQ], F16, kind="ExternalInput")
    wkv = nc.dram_tensor("wkv", [128, NDT * 128], F16, kind="ExternalInput")
    wo = nc.dram_tensor("wo", [128, 2 * DIM], F16, kind="ExternalInput")
    bqd = nc.dram_tensor("bq", [CQ], F32, kind="ExternalInput")
    bkvd = nc.dram_tensor("bkv", [128], F32, kind="ExternalInput")
    ccd = nc.dram_tensor("cc", [128, S], F16, kind="ExternalInput")
    ssd = nc.dram_tensor("ssgn", [128, S], F16, kind="ExternalInput")
    trid = nc.dram_tensor("triband", [128, 2, 128], F16, kind="ExternalInput")
    if use_mask:
        maskt = nc.dram_tensor("maskt", [S, S], F16, kind="ExternalInput")
    partial = nc.dram_tensor("partial", [S, DIM], F16, kind="ExternalOutput")

    with tile.TileContext(nc) as tc:
        with tc.tile_pool(name="persist", bufs=1) as pp:
            # --- persistent inputs, spread across DMA queues ---
            # sync: biases then the x stream (phase-1 critical path)
            bq_sb = pp.tile([128, 2], F32)
            nc.sync.dma_start(bq_sb[:], bqd.ap().rearrange("(t p) -> p t", p=128))
            bkv_sb = pp.tile([128, 1], F32)
            nc.sync.dma_start(bkv_sb[:], bkvd.ap()[:, None])
            # sync: projection weights (needed by the very first matmuls)
            wq_sb = pp.tile([128, NDT, CQ], F16)
            nc.sync.dma_start(wq_sb[:], wq.ap().rearrange("p (a c) -> p a c", c=CQ))
            wkv_sb = pp.tile([128, NDT, 128], F16)
            nc.sync.dma_start(wkv_sb[:], wkv.ap().rearrange("p (a c) -> p a c", c=128))
            # gpsimd: rope tables + small constants (needed ~10us in)
            cc_sb = pp.tile([128, S], F16)
            nc.gpsimd.dma_start(cc_sb[:], ccd[:])
            ss_sb = pp.tile([128, S], F16)
            nc.gpsimd.dma_start(ss_sb[:], ssd[:])
            tri_sb = pp.tile([128, 2, 128], F16)
            nc.gpsimd.dma_start(tri_sb[:], trid[:])
            ones65 = pp.tile([65, 64], F16, name="ones65")
            nc.gpsimd.memset(ones65[64:65, :], 1.0)
            # scalar: out-projection weights (phase 2 only)
            wo_sb = pp.tile([128, 2, DIM], F16)
            nc.scalar.dma_start(wo_sb[:], wo.ap().rearrange("p (a e) -> p a e", e=DIM))
            if use_mask:
                mask_sb = pp.tile([128, NKT, S], F16)
                nc.scalar.dma_start(mask_sb[:],
                                    maskt.ap().rearrange("(a p) q -> p a q", p=128))

            # persistent activations
            qT = [pp.tile([128, S], F16, tag=f"qt{i}", name=f"qt{i}") for i in range(2)]
            kT = pp.tile([128, S], F16)           # rows 0:64 and 64:128 both = roped k
            v_sb = pp.tile([128, NKT, 65], F16)   # [kpos, ktile, ch + ones]
            nc.gpsimd.memset(v_sb[:, :, 64:65], 1.0)
            attn = [[pp.tile([128, QT], F16, tag=f"attn{c}_{t}", name=f"attn{c}_{t}")
                     for t in range(NQT)] for c in range(2)]

            # ------- Phase 1: QKV projections + fused RoPE + v transpose -------
            with tc.tile_pool(name="xstream", bufs=2) as xp, \
                 tc.tile_pool(name="qkvps", bufs=2, space="PSUM") as qps, \
                 tc.tile_pool(name="rawp", bufs=2) as rawp, \
                 tc.tile_pool(name="vstg", bufs=3) as vstg, \
                 tc.tile_pool(name="rope", bufs=3) as rp:

                def rope(raw_ap, out_ap, rows, ssl, swq):
                    sw = rp.tile([128, ST], F16, tag="ropesw")
                    t1 = rp.tile([128, ST], F16, tag="ropet1")
                    for b in rows:
                        swq.dma_start(sw[b:b + 32, :], raw_ap[b + 32:b + 64, :])
                        swq.dma_start(sw[b + 32:b + 64, :], raw_ap[b:b + 32, :])
                    lo, hi = rows[0], rows[-1] + 64
                    nc.vector.tensor_mul(t1[lo:hi, :], raw_ap[lo:hi, :], cc_sb[lo:hi, ssl])
                    nc.vector.tensor_mul(sw[lo:hi, :], sw[lo:hi, :], ss_sb[lo:hi, ssl])
                    nc.vector.tensor_add(out_ap[lo:hi, ssl], t1[lo:hi, :], sw[lo:hi, :])

                for st in range(NST):
                    ssl = ts(st, ST)
                    xt = xp.tile([128, NDT, ST], F16, tag="xt")
                    xr = xT.ap()[st].rearrange("p (a s) -> p a s", s=ST)
                    for half in range(2):
                        nc.sync.dma_start(xt[:, ts(half, 8), :], xr[:, ts(half, 8), :])
                    pq0 = qps.tile([128, ST], F32, tag="q0")
                    pq1 = qps.tile([128, ST], F32, tag="q1")
                    pkv = qps.tile([128, ST], F32, tag="kv")
                    for kt in range(NDT):
                        st_flag, sp_flag = kt == 0, kt == NDT - 1
                        nc.tensor.matmul(pq0[:], wq_sb[:, kt, 0:128], xt[:, kt, :],
                                         start=st_flag, stop=sp_flag)
                        nc.tensor.matmul(pq1[:], wq_sb[:, kt, 128:256], xt[:, kt, :],
                                         start=st_flag, stop=sp_flag)
                        nc.tensor.matmul(pkv[:], wkv_sb[:, kt, :], xt[:, kt, :],
                                         start=st_flag, stop=sp_flag)
                    q0_raw = rawp.tile([128, ST], F16, tag="q0r")
                    q1_raw = rawp.tile([128, ST], F16, tag="q1r")
                    kv_raw = rawp.tile([128, ST], F16, tag="kvr")
                    nc.scalar.activation(q0_raw[:], pq0[:],
                                         mybir.ActivationFunctionType.Identity,
                                         bias=bq_sb[:, 0:1])
                    nc.scalar.activation(q1_raw[:], pq1[:],
                                         mybir.ActivationFunctionType.Identity,
                                         bias=bq_sb[:, 1:2])
                    nc.scalar.activation(kv_raw[:], pkv[:],
                                         mybir.ActivationFunctionType.Identity,
                                         bias=bkv_sb[:, 0:1])
                    # keep the sync engine free for the x stream: swap DMAs
                    # that wait on compute go on gpsimd/scalar queues
                    rope(q0_raw[:], qT[0][:], [0, 64], ssl, nc.gpsimd)
                    rope(q1_raw[:], qT[1][:], [0, 64], ssl, nc.scalar)
                    rope(kv_raw[:], kT[:], [64], ssl, nc.gpsimd)
                    nc.gpsimd.dma_start(kT[0:64, ssl], kT[64:128, ssl])
                    for vc in range(ST // 128):
                        j = (st * ST) // 128 + vc
                        # XBAR transpose misaddresses offset outputs; stage at
                        # offset 0 then copy into the strided v slot
                        stg = vstg.tile([128, 64], F16, tag="stg")
                        nc.scalar.dma_start(stg[:], kv_raw[0:64, ts(vc, 128)],
                                            transpose=True)
                        nc.vector.tensor_copy(v_sb[:, j, 0:64], stg[:])

            # ------- Phase 2+3 fused: attention with interleaved out-proj -------
            with tc.tile_pool(name="scps", bufs=2, space="PSUM") as scps, \
                 tc.tile_pool(name="avps", bufs=2, space="PSUM") as avps, \
                 tc.tile_pool(name="opsp", bufs=2, space="PSUM") as opsp, \
                 tc.tile_pool(name="ptp", bufs=3) as ptp, \
                 tc.tile_pool(name="nrm", bufs=2) as nrm, \
                 tc.tile_pool(name="osb", bufs=4) as osb:
                # deferred norm/out-proj emission: flush inside the NEXT pass's
                # j-loop so the PE chews on fresh scores while DVE runs the
                # serialized normalize chain of the previous pass
                pending = []

                def flush():
                    for fn in pending:
                        fn()
                    pending.clear()

                def make_norm(t, hp, aps):
                    def emit():
                        bcp = scps.tile([128, 2, QT], F32, tag="sc", name="bc")
                        for hh in range(2):
                            avsb = nrm.tile([65, QT], F32, tag="avsb")
                            nc.vector.tensor_copy(avsb[:], aps[hh][:])
                            # NB: single-partition custom-DVE ops silently
                            # no-op; run on the full 65-partition tile
                            rc = nrm.tile([65, QT], F32, tag="rc")
                            nc.vector.reciprocal_approx_fast(rc[:, :], avsb[:, :])
                            rch = nrm.tile([65, QT], F16, tag="rch")
                            nc.vector.tensor_copy(rch[64:65, :], rc[64:65, :])
                            nc.tensor.matmul(bcp[0:64, hh, :], ones65[64:65, :],
                                             rch[64:65, :], start=True, stop=True)
                            if hh == 0:
                                nc.vector.tensor_mul(attn[hp][t][0:64, :],
                                                     avsb[0:64, :],
                                                     bcp[0:64, hh, :])
                            else:
                                tb = nrm.tile([64, QT], F16, tag="tb")
                                nc.vector.tensor_mul(tb[:], avsb[0:64, :],
                                                     bcp[0:64, hh, :])
                                nc.gpsimd.dma_start(attn[hp][t][64:128, :], tb[:])
                    return emit

                def make_outproj(t):
                    def emit():
                        for sl in range(4):
                            ssub = 4 * t + sl
                            for et in range(4):
                                pps = opsp.tile([128, 512], F32, tag="op")
                                for ct in range(2):
                                    nc.tensor.matmul(pps[:],
                                                     attn[ct][t][:, ts(sl, 128)],
                                                     wo_sb[:, ct, ts(et, 512)],
                                                     start=(ct == 0), stop=(ct == 1))
                                ot = osb.tile([128, 512], F16, tag="ot")
                                if (4 * sl + et) % 3 == 2:
                                    nc.scalar.activation(
                                        ot[:], pps[:],
                                        mybir.ActivationFunctionType.Copy)
                                else:
                                    nc.vector.tensor_copy(ot[:], pps[:])
                                nc.sync.dma_start(
                                    partial[ts(ssub, 128), ts(et, 512)], ot[:])
                    return emit

                for t in range(NQT):
                    tsl = ts(t, QT)
                    n_k = 4 * (t + 1) if causal else NKT
                    for hp in range(2):
                        aps = [avps.tile([65, QT], F32, tag="av", name=f"av{t}_{hp}_{hh}")
                               for hh in range(2)]
                        pts = {}
                        c0s = {}
                        for j in range(n_k):
                            jsl = ts(j, 128)
                            c0 = 128 * (j - 4 * t) if (causal and j >= 4 * t) else 0
                            c0s[j] = c0
                            sc = scps.tile([128, 2, QT], F32, tag="sc", name="sc")
                            for half, base in ((0, 0), (1, 64)):
                                nc.tensor.matmul(sc[:, half, c0:],
                                                 kT[base:base + 64, jsl],
                                                 qT[hp][base:base + 64, tsl][:, c0:],
                                                 start=True, stop=True)
                            pt = ptp.tile([128, 2, QT], F16, tag="pt")
                            nc.scalar.activation(pt[:, :, c0:], sc[:, :, c0:],
                                                 mybir.ActivationFunctionType.Exp,
                                                 scale=0.125)
                            if causal and j >= 4 * t:
                                nc.gpsimd.tensor_mul(pt[:, :, c0:c0 + 128],
                                                     pt[:, :, c0:c0 + 128],
                                                     tri_sb[:])
                            if use_mask:
                                for hh in range(2):
                                    nc.vector.tensor_mul(pt[:, hh, :], pt[:, hh, :],
                                                         mask_sb[:, j, tsl])
                            pts[j] = pt
                            if j == 1:
                                flush()
                            # PV one j behind scores keeps the PE fed during exp
                            if j > 0:
                                jp = j - 1
                                for hh in range(2):
                                    nc.tensor.matmul(aps[hh][:, c0s[jp]:],
                                                     v_sb[:, jp, :],
                                                     pts[jp][:, hh, c0s[jp]:],
                                                     start=(jp == 0), stop=False)
                                del pts[jp]
                        jp = n_k - 1
                        for hh in range(2):
                            nc.tensor.matmul(aps[hh][:, c0s[jp]:], v_sb[:, jp, :],
                                             pts[jp][:, hh, c0s[jp]:],
                                             start=(jp == 0), stop=True)
                        pending.append(make_norm(t, hp, aps))
                    pending.append(make_outproj(t))
                flush()
                if dump:
                    for i in range(2):
                        nc.sync.dma_start(qTd[i], qT[i][:])
                    nc.sync.dma_start(kTd[:], kT[:])
                    nc.sync.dma_start(vd[:], v_sb[:])
                    for c in range(2):
                        for t in range(NQT):
                            nc.sync.dma_start(attnd[c, t], attn[c][t][:])

    nc.compile()
    return nc


_CACHE = {}
TRACE = False
LAST_EXEC_NS = None
LAST_RES = None


def _get(causal, use_mask):
    key = (causal, use_mask)
    if key not in _CACHE:
        _CACHE[key] = _build(causal, use_mask)
    return _CACHE[key]


def _perm_eo(w):
    # de-interleave channel pairs per 64-col head block: [evens, odds]
    cols = np.concatenate([np.arange(0, 64, 2), np.arange(1, 64, 2)])
    return w[..., cols]


def kernel(**inputs):
    x = np.asarray(inputs["x"], dtype=np.float32)
    fc = np.asarray(inputs["freqs_cos"], dtype=np.float32)
    fs = np.asarray(inputs["freqs_sin"], dtype=np.float32)
    mask = np.asarray(inputs["mask"])
    Wq = np.asarray(inputs["Wq"], dtype=np.float32)
    bq = np.asarray(inputs["bq"], dtype=np.float32)
    Wk = np.asarray(inputs["Wk"], dtype=np.float32)
    bk = np.asarray(inputs["bk"], dtype=np.float32)
    Wv = np.asarray(inputs["Wv"], dtype=np.float32)
    bv = np.asarray(inputs["bv"], dtype=np.float32)
    Wo = np.asarray(inputs["Wo"], dtype=np.float32)
    bo = np.asarray(inputs["bo"], dtype=np.float32)

    m2 = mask.reshape(S, S)
    if (m2 == 1).all():
        causal, use_mask = False, False
    elif np.array_equal(m2 != 0, np.tril(np.ones((S, S), dtype=bool))):
        causal, use_mask = True, False
    else:
        causal, use_mask = False, True
    nc = _get(causal, use_mask)

    f16 = np.float16

    def swiz(w, inner):
        # [A*128, inner] -> [128, A*inner]: row a*128+p becomes (p, a)
        a = w.shape[0] // 128
        return np.ascontiguousarray(
            w.reshape(a, 128, inner).transpose(1, 0, 2).reshape(128, a * inner),
            dtype=f16)

    # x: [DIM, S] -> [NST, 128, NDT*ST] with per-partition contiguous s-tiles
    xTf = x[0].T.astype(f16)              # (DIM, S)
    xT = np.ascontiguousarray(
        xTf.reshape(NDT, 128, NST, ST).transpose(2, 1, 0, 3).reshape(
            NST, 128, NDT * ST))
    cosT = fc.T  # (32, S)
    sinT = fs.T
    cc = np.ascontiguousarray(np.tile(cosT, (4, 1)), dtype=f16)
    ssgn = np.ascontiguousarray(
        np.concatenate([-sinT, sinT, -sinT, sinT], axis=0), dtype=f16)
    kl = np.arange(128)[:, None]
    qq = np.arange(128)[None, :]
    band = (qq >= kl).astype(f16)                  # [128,128] step
    tri = np.ascontiguousarray(
        np.broadcast_to(band[:, None, :], (128, 2, 128)), dtype=f16)

    Wq_h = Wq.reshape(DIM, H, D)
    bq_h = bq.reshape(H, D)
    Wk_h = Wk.reshape(DIM, KVH, D)
    bk_h = bk.reshape(KVH, D)

    in_maps = []
    for c in range(NCORES):
        hs = slice(HQ * c, HQ * (c + 1))
        wq_c = _perm_eo(Wq_h[:, hs, :]).reshape(DIM, CQ)
        bq_c = _perm_eo(bq_h[hs, :]).reshape(CQ)
        wk_c = _perm_eo(Wk_h[:, c, :])
        bk_c = _perm_eo(bk_h[c, :])
        wv_c = Wv[:, 64 * c:64 * (c + 1)]
        bv_c = bv[64 * c:64 * (c + 1)]
        wkv_c = np.concatenate([wv_c, wk_c], axis=1)
        bkv_c = np.concatenate([bv_c, bk_c]).astype(np.float32)
        wo_c = Wo[CQ * c:CQ * (c + 1), :]
        im = {
            "xT": xT, "wq": swiz(wq_c, CQ),
            "wkv": swiz(wkv_c, 128), "wo": swiz(wo_c, DIM),
            "bq": np.ascontiguousarray(bq_c, dtype=np.float32),
            "bkv": np.ascontiguousarray(bkv_c), "cc": cc,
            "ssgn": ssgn, "triband": tri,
        }
        if use_mask:
            im["maskt"] = np.ascontiguousarray(m2.T.astype(f16))
        in_maps.append(im)

    global LAST_EXEC_NS, LAST_RES
    res = run_bass_kernel_spmd(nc, in_maps, core_ids=list(range(NCORES)), trace=TRACE)
    LAST_EXEC_NS = res.exec_time_ns
    LAST_RES = res
    out = np.zeros((S, DIM), dtype=np.float32)
    for rr in res.results:
        out += rr["partial"].astype(np.float32)
    out += bo
    return out.reshape(1, S, DIM)
